# Initial kernel scaffold
#
"""Trainium2 Bass kernel for nn_DiscreteSequenceModel (GRU rollout).

Math (see reference): h0 = y0 @ enc_w.T + enc_b, then 512 sequential GRU
steps with input == hidden (ts values are unused by the math; only
len(ts) == T matters), emitting pred_t = h_t @ dec_w.T + dec_b before each
update.

Sharding: data-parallel over batch B=256 across 8 cores (32 rows/core),
weights replicated.  Per core, each step contracts h (K=1024, 8 k-tiles of
128) against a packed weight matrix with 4352 output columns
(r|z|hn|pred|inn per feature-group).  To keep the 128x128 PE array full
with only 32 batch rows, we column-tile the array into 4 groups of 32:
every group holds the same stationary h-tile but streams a different
feature-chunk of the weights, so the array does 4 concurrent 32-wide
matmuls (128 effective rows).  Gate outputs land in PSUM as
[4 groups x 32 batch, features], which is also the layout the elementwise
GRU math wants (128 full partitions).  The next step's stationary h^T
tiles are produced with two PE transposes of the new hidden state.

Biases (gate bias, bias_n, dec_b) are folded into the contraction as a
9th K=1 "tile" (ones vector x bias row).  Matmuls run in float32r (full
fp32 storage; replicated-weight PE path, 1 cycle/row at N>=256).
"""

import numpy as np

import concourse.bass as bass
import concourse.tile as tile
from concourse import mybir
from concourse.bass_utils import run_bass_kernel_spmd
from concourse.masks import make_identity

B, T, H, D = 256, 512, 1024, 256
NCORES = 8
BL = B // NCORES            # 32 batch rows per core
NG = 4                      # PE column-tile groups
FC = H // NG                # 256 gate features per group
PC = D // NG                # 64 decoder features per group
GW = 4 * FC + PC            # 1088 packed weight cols per group
KT = H // 128               # 8 k-tiles
WSW = KT * NG * GW          # SBUF weight width = 8*4352

# free-dim offsets inside one group's 1088-wide strip
RZ0, RZ1 = 0, 2 * FC        # [r 256 | z 256]
HP0, HP1 = 2 * FC, 3 * FC + PC   # [hn 256 | pred 64]
IN0, IN1 = 3 * FC + PC, GW       # [inn 256]

F32 = mybir.dt.float32
F32R = mybir.dt.float32r
AFT = mybir.ActivationFunctionType


def _emit(tc, nc, wp, br, ep, eb, y0t, preds, steps, unroll):
    assert steps % unroll == 0 and unroll % 2 == 0
    import contextlib

    with contextlib.ExitStack() as ctx:
        const = ctx.enter_context(tc.tile_pool(name="const", bufs=1))
        WS = const.tile([128, WSW], F32)
        BR = const.tile([1, NG * GW], F32)
        ES = const.tile([128, 2 * H], F32)
        EB = const.tile([1, H], F32)
        Y0T = const.tile([128, 2 * BL], F32)
        ID = const.tile([128, 128], F32)
        ONES = const.tile([1, BL], F32)
        H0 = const.tile([128, FC], F32)
        H1 = const.tile([128, FC], F32)

        for k in range(KT):
            nc.sync.dma_start(WS[:, k * NG * GW:(k + 1) * NG * GW], wp[k])
        nc.sync.dma_start(BR[:], br[:])
        for k in range(2):
            nc.sync.dma_start(ES[:, k * H:(k + 1) * H], ep[k])
            nc.sync.dma_start(Y0T[:, k * BL:(k + 1) * BL],
                              y0t[128 * k:128 * (k + 1), :])
        nc.sync.dma_start(EB[:], eb[:])
        make_identity(nc, ID[:])
        nc.gpsimd.memset(ONES[:], 1.0)

        ps = ctx.enter_context(tc.tile_pool(name="ps", bufs=1, space="PSUM"))
        ps_rz = ps.tile([128, 2 * FC], F32)
        ps_hp = ps.tile([128, FC + PC], F32)
        ps_in = ps.tile([128, FC], F32)
        ps_t1 = ps.tile([128, 128], F32)
        ps_t2 = ps.tile([128, 128], F32)

        sb = ctx.enter_context(tc.tile_pool(name="sb", bufs=2))

        preds_f = preds.rearrange("b t d -> b (t d)")

        # ---- encoder: h0 in gate layout [32j+b, f_local] ----
        for j in range(NG):
            for k in range(2):
                nc.tensor.matmul(
                    ps_rz[32 * j:32 * j + 32, 0:FC],
                    Y0T[:, k * BL:(k + 1) * BL].bitcast(F32R),
                    ES[:, k * H + j * FC:k * H + (j + 1) * FC].bitcast(F32R),
                    start=(k == 0), stop=False, tile_position=(0, 32 * j))
            nc.tensor.matmul(
                ps_rz[32 * j:32 * j + 32, 0:FC],
                ONES[0:1, :].bitcast(F32R),
                EB[0:1, j * FC:(j + 1) * FC].bitcast(F32R),
                start=False, stop=True, tile_position=(0, 32 * j))
        nc.scalar.copy(H0[:], ps_rz[:, 0:FC])

        def step_body(tv, parity):
            hc = (H0 if parity == 0 else H1)[:]
            hnew = (H1 if parity == 0 else H0)[:]

            # h^T k-tiles for this step via two PE transposes
            nc.tensor.transpose(ps_t1[:], hc[:, 0:128], ID[:])
            nc.tensor.transpose(ps_t2[:], hc[:, 128:256], ID[:])
            hTe = sb.tile([128, 128], F32, tag="hTe")
            hTo = sb.tile([128, 128], F32, tag="hTo")
            nc.scalar.copy(hTe[:], ps_t1[:])
            nc.vector.tensor_copy(hTo[:], ps_t2[:])

            def mm(out_ap, k, j, c0, c1, start, stop):
                if k < KT:
                    src = hTe if k % 2 == 0 else hTo
                    m = k // 2
                    lhsT = src[:, m * 32:(m + 1) * 32]
                    rhs = WS[:, k * NG * GW + j * GW + c0:
                             k * NG * GW + j * GW + c1]
                else:  # bias "k-tile": ones x bias row
                    lhsT = ONES[0:1, :]
                    rhs = BR[0:1, j * GW + c0:j * GW + c1]
                nc.tensor.matmul(out_ap, lhsT.bitcast(F32R), rhs.bitcast(F32R),
                                 start=start, stop=stop,
                                 tile_position=(0, 32 * j))

            # pass A: r|z pre-activations
            for k in range(KT + 1):
                for j in range(NG):
                    mm(ps_rz[32 * j:32 * j + 32, :], k, j, RZ0, RZ1,
                       k == 0, k == KT)

            r = sb.tile([128, FC], F32, tag="r")
            z = sb.tile([128, FC], F32, tag="z")
            omz = sb.tile([128, FC], F32, tag="omz")
            zh = sb.tile([128, FC], F32, tag="zh")
            nc.scalar.activation(r[:], ps_rz[:, 0:FC], AFT.Sigmoid)
            nc.scalar.activation(z[:], ps_rz[:, FC:2 * FC], AFT.Sigmoid)
            # 1 - sigmoid(x) == sigmoid(-x)
            nc.scalar.activation(omz[:], ps_rz[:, FC:2 * FC], AFT.Sigmoid,
                                 scale=-1.0)
            nc.vector.tensor_mul(zh[:], z[:], hc)

            # pass B: hn|pred and inn
            for k in range(KT + 1):
                for j in range(NG):
                    mm(ps_hp[32 * j:32 * j + 32, :], k, j, HP0, HP1,
                       k == 0, k == KT)
                    mm(ps_in[32 * j:32 * j + 32, :], k, j, IN0, IN1,
                       k == 0, k == KT)

            pred_sb = sb.tile([128, PC], F32, tag="pred")
            nc.scalar.copy(pred_sb[:], ps_hp[:, FC:FC + PC])

            v = sb.tile([128, FC], F32, tag="v")
            w2 = sb.tile([128, FC], F32, tag="w2")
            nt = sb.tile([128, FC], F32, tag="nt")
            t4 = sb.tile([128, FC], F32, tag="t4")
            nc.vector.tensor_mul(v[:], r[:], ps_hp[:, 0:FC])
            nc.vector.tensor_add(w2[:], v[:], ps_in[:])
            nc.scalar.activation(nt[:], w2[:], AFT.Tanh)
            nc.vector.tensor_mul(t4[:], nt[:], omz[:])
            nc.vector.tensor_add(hnew, t4[:], zh[:])

            for j in range(NG):
                nc.sync.dma_start(
                    preds_f[:, bass.ds(tv * D + j * PC, PC)],
                    pred_sb[32 * j:32 * j + 32, :])

        with tc.For_i(0, steps, unroll,
                      hint_engines=(mybir.EngineType.PE,)) as iv:
            for s in range(unroll):
                step_body(iv + s, s % 2)


_CACHE = {}


def _get_nc(steps, unroll):
    key = (steps, unroll)
    if key in _CACHE:
        return _CACHE[key]
    nc = bass.Bass("TRN2", target_bir_lowering=False, debug=False,
                   enable_asserts=False, num_devices=NCORES)
    wp = nc.dram_tensor("wp", [KT, 128, NG * GW], F32,
                        kind="ExternalInput").ap()
    br = nc.dram_tensor("br", [1, NG * GW], F32, kind="ExternalInput").ap()
    ep = nc.dram_tensor("ep", [2, 128, H], F32, kind="ExternalInput").ap()
    eb = nc.dram_tensor("eb", [1, H], F32, kind="ExternalInput").ap()
    y0t = nc.dram_tensor("y0t", [D, BL], F32, kind="ExternalInput").ap()
    preds = nc.dram_tensor("preds", [BL, steps, D], F32,
                           kind="ExternalOutput").ap()
    with tile.TileContext(nc) as tc:
        _emit(tc, nc, wp, br, ep, eb, y0t, preds, steps, unroll)
    _CACHE[key] = nc
    return nc


def _pack(y0_batch, enc_w, enc_b, w_ih, w_hh, bias, bias_n, dec_w, dec_b):
    f = lambda x: np.ascontiguousarray(np.asarray(x, dtype=np.float32))
    y0_batch, enc_w, enc_b = f(y0_batch), f(enc_w), f(enc_b)
    w_ih, w_hh, bias, bias_n = f(w_ih), f(w_hh), f(bias), f(bias_n)
    dec_w, dec_b = f(dec_w), f(dec_b)

    W_r = w_ih[0:H] + w_hh[0:H]
    W_z = w_ih[H:2 * H] + w_hh[H:2 * H]
    W_ni = w_ih[2 * H:3 * H]
    W_nh = w_hh[2 * H:3 * H]

    wcols, bcols = [], []
    for j in range(NG):
        f0, f1 = j * FC, (j + 1) * FC
        p0, p1 = j * PC, (j + 1) * PC
        wcols += [W_r[f0:f1].T, W_z[f0:f1].T, W_nh[f0:f1].T,
                  dec_w[p0:p1].T, W_ni[f0:f1].T]
        bcols += [bias[f0:f1], bias[H + f0:H + f1], bias_n[f0:f1],
                  dec_b[p0:p1], bias[2 * H + f0:2 * H + f1]]
    wp = np.ascontiguousarray(
        np.concatenate(wcols, axis=1).reshape(KT, 128, NG * GW))
    br = np.ascontiguousarray(np.concatenate(bcols)[None, :])

    ep = np.ascontiguousarray(
        np.concatenate([enc_w[j * FC:(j + 1) * FC, :].T for j in range(NG)],
                       axis=1).reshape(2, 128, H))
    eb = np.ascontiguousarray(
        np.concatenate([enc_b[j * FC:(j + 1) * FC] for j in range(NG)])[None, :])

    shared = dict(wp=wp, br=br, ep=ep, eb=eb)
    in_maps = []
    for c in range(NCORES):
        y0t = np.ascontiguousarray(y0_batch[c * BL:(c + 1) * BL].T)
        in_maps.append(dict(shared, y0t=y0t))
    return in_maps


def _run(inputs, steps=T, unroll=8, **run_kwargs):
    in_maps = _pack(
        inputs["y0_batch"], inputs["enc_w"], inputs["enc_b"], inputs["w_ih"],
        inputs["w_hh"], inputs["bias"], inputs["bias_n"], inputs["dec_w"],
        inputs["dec_b"])
    nc = _get_nc(steps, unroll)
    res = run_bass_kernel_spmd(nc, in_maps, core_ids=list(range(NCORES)),
                               **run_kwargs)
    out = np.concatenate([r["preds"] for r in res.results], axis=0)
    return out, res


def kernel(ts=None, y0_batch=None, enc_w=None, enc_b=None, w_ih=None,
           w_hh=None, bias=None, bias_n=None, dec_w=None, dec_b=None):
    steps = int(np.asarray(ts).shape[0]) if ts is not None else T
    out, _ = _run(dict(y0_batch=y0_batch, enc_w=enc_w, enc_b=enc_b,
                       w_ih=w_ih, w_hh=w_hh, bias=bias, bias_n=bias_n,
                       dec_w=dec_w, dec_b=dec_b), steps=steps)
    return out


# revision 17
# speedup vs baseline: 1.5971x; 1.5971x over previous
"""Trainium2 Bass kernel for nn_DiscreteSequenceModel (GRU rollout).

Math (see reference): h0 = y0 @ enc_w.T + enc_b, then T=512 sequential GRU
steps with input == hidden (ts values are unused by the math; only len(ts)
matters), emitting pred_t = h_t @ dec_w.T + dec_b before each update.

Sharding: data-parallel over batch B=256 across 8 cores (32 rows/core),
weights replicated.  Per core, each step contracts h (K=1024, 8 k-tiles of
128) against a packed weight matrix with 4352 output columns
(r|z|hn|pred|inn per feature-group).  To keep the 128x128 PE array full
with only 32 batch rows, we column-tile the array into 4 groups of 32:
every group holds the same stationary h-tile but streams a different
feature-chunk of the weights, so the array does 4 concurrent 32-wide
matmuls (128 effective rows).  Gate outputs land in PSUM as
[4 groups x 32 batch, features], which is also the layout the elementwise
GRU math wants (128 full partitions).  The next step's stationary h^T
tiles are produced with two PE transposes of the new hidden state.

Biases (gate bias, bias_n, dec_b) are folded into the contraction as a
9th K=1 "tile" (ones vector x bias row).  Matmuls run in fp16 (the PE's 4-byte float path rejects column tiling;
fp16 keeps a tf32-grade 10-bit mantissa and our values are small), PSUM
accumulation in fp32, and the carried hidden state stays fp32 — only the
matmul operands round.
"""

import numpy as np

import concourse.bacc as bacc
import concourse.bass as bass
import concourse.tile as tile
from concourse import mybir
from concourse.bass_utils import run_bass_kernel_spmd

B, T, H, D = 256, 512, 1024, 256
NCORES = 8
BL = B // NCORES            # 32 batch rows per core
NG = 4                      # PE column-tile groups
FC = H // NG                # 256 gate features per group
PC = D // NG                # 64 decoder features per group
GW = 4 * FC + PC            # 1088 packed weight cols per group
KT = H // 128               # 8 k-tiles

# free-dim offsets inside one group's 1088-wide strip
RZ0, RZ1 = 0, 2 * FC             # [r 256 | z 256]
HP0, HP1 = 2 * FC, 3 * FC + PC   # [hn 256 | pred 64]
IN0, IN1 = 3 * FC + PC, GW       # [inn 256]

# blob column layout (one [128, BLOBW] fp32 tensor holds every constant)
OFF_WS = 0                       # weights: [p, k*4352 + j*1088 + c]
OFF_ES = OFF_WS + KT * NG * GW   # encoder weights: [p, k*1024 + j*256 + c]
OFF_Y0T = OFF_ES + 2 * H         # y0^T k-tiles: [p, k*32 + b]
OFF_BR = OFF_Y0T + 2 * BL        # row 0: packed gate/pred bias row (4352)
OFF_EB = OFF_BR + NG * GW        # row 0: packed encoder bias row (1024)
OFF_ID = OFF_EB + H              # 128x128 identity (for PE transpose)
OFF_ONES = OFF_ID + 128          # row 0: 32 ones (bias k-tile stationary)
BLOBW = OFF_ONES + BL

F32 = mybir.dt.float32
FP16 = mybir.dt.float16
AFT = mybir.ActivationFunctionType


def _emit(tc, nc, blob, idf, preds, steps, unroll):
    assert steps % unroll == 0 and unroll % 2 == 0
    import contextlib

    with contextlib.ExitStack() as ctx:
        const = ctx.enter_context(tc.tile_pool(name="const", bufs=1))
        C = const.tile([128, BLOBW], FP16)
        H0 = const.tile([128, FC], F32)
        H1 = const.tile([128, FC], F32)

        IDT = const.tile([128, 128], F32)
        nc.sync.dma_start(C[:], blob[:])
        nc.sync.dma_start(IDT[:], idf[:])

        def ws(k, j, c0, c1):
            o = OFF_WS + k * NG * GW + j * GW
            return C[:, o + c0:o + c1]

        ID = IDT[:]
        ONES = C[0:1, OFF_ONES:OFF_ONES + BL]

        ps = ctx.enter_context(tc.tile_pool(name="ps", bufs=1, space="PSUM"))
        ps_rz = ps.tile([128, 2 * FC], F32)
        ps_hp = ps.tile([128, FC + PC], F32)
        ps_in = ps.tile([128, FC], F32)
        ps_t1 = ps.tile([128, 128], F32)
        ps_t2 = ps.tile([128, 128], F32)

        sb = ctx.enter_context(tc.tile_pool(name="sb", bufs=2))

        # ---- encoder: h0 in gate layout [32j+b, f_local] ----
        for j in range(NG):
            for k in range(2):
                nc.tensor.matmul(
                    ps_rz[32 * j:32 * j + 32, 0:FC],
                    C[:, OFF_Y0T + k * BL:OFF_Y0T + (k + 1) * BL],
                    C[:, OFF_ES + k * H + j * FC:
                      OFF_ES + k * H + (j + 1) * FC],
                    start=(k == 0), stop=False, skip_group_check=True,
                    tile_position=(0, 32 * j))
            nc.tensor.matmul(
                ps_rz[32 * j:32 * j + 32, 0:FC],
                ONES,
                C[0:1, OFF_EB + j * FC:OFF_EB + (j + 1) * FC],
                start=False, stop=True, skip_group_check=True,
                tile_position=(0, 32 * j))
        nc.scalar.copy(H0[:], ps_rz[:, 0:FC])

        def step_body(tv, sub, stage):
            parity = sub % 2
            hc = (H0 if parity == 0 else H1)[:]
            hnew = (H1 if parity == 0 else H0)[:]

            # h^T k-tiles for this step via two PE transposes
            nc.tensor.transpose(ps_t1[:], hc[:, 0:128], ID)
            nc.tensor.transpose(ps_t2[:], hc[:, 128:256], ID)
            hTe = sb.tile([128, 128], FP16, tag="hTe")
            hTo = sb.tile([128, 128], FP16, tag="hTo")
            nc.scalar.copy(hTe[:], ps_t1[:])
            nc.vector.tensor_copy(hTo[:], ps_t2[:])

            def mm(out_ap, k, j, c0, c1, start, stop):
                if k < KT:
                    src = hTe if k % 2 == 0 else hTo
                    m = k // 2
                    lhsT = src[:, m * 32:(m + 1) * 32]
                    rhs = ws(k, j, c0, c1)
                else:  # bias "k-tile": ones x bias row
                    lhsT = ONES
                    rhs = C[0:1, OFF_BR + j * GW + c0:OFF_BR + j * GW + c1]
                nc.tensor.matmul(out_ap, lhsT, rhs,
                                 start=start, stop=stop,
                                 skip_group_check=True,
                                 tile_position=(0, 32 * j))

            # pass A: r|z pre-activations
            for k in range(KT + 1):
                for j in range(NG):
                    mm(ps_rz[32 * j:32 * j + 32, :], k, j, RZ0, RZ1,
                       k == 0, k == KT)

            r = sb.tile([128, FC], F32, tag="r")
            z = sb.tile([128, FC], F32, tag="z")
            omz = sb.tile([128, FC], F32, tag="omz")
            zh = sb.tile([128, FC], F32, tag="zh")
            nc.scalar.activation(r[:], ps_rz[:, 0:FC], AFT.Sigmoid)
            nc.scalar.activation(z[:], ps_rz[:, FC:2 * FC], AFT.Sigmoid)
            # 1 - sigmoid(x) == sigmoid(-x)
            nc.scalar.activation(omz[:], ps_rz[:, FC:2 * FC], AFT.Sigmoid,
                                 scale=-1.0)
            nc.vector.tensor_mul(zh[:], z[:], hc)

            # pass B: hn|pred and inn
            for k in range(KT + 1):
                for j in range(NG):
                    mm(ps_hp[32 * j:32 * j + 32, :], k, j, HP0, HP1,
                       k == 0, k == KT)
                    mm(ps_in[32 * j:32 * j + 32, :], k, j, IN0, IN1,
                       k == 0, k == KT)

            nc.scalar.copy(stage[:, sub * PC:(sub + 1) * PC],
                           ps_hp[:, FC:FC + PC])

            v = sb.tile([128, FC], F32, tag="v")
            w2 = sb.tile([128, FC], F32, tag="w2")
            nt = sb.tile([128, FC], F32, tag="nt")
            t4 = sb.tile([128, FC], F32, tag="t4")
            nc.vector.tensor_mul(v[:], r[:], ps_hp[:, 0:FC])
            nc.vector.tensor_add(w2[:], v[:], ps_in[:])
            nc.scalar.activation(nt[:], w2[:], AFT.Tanh)
            nc.vector.tensor_mul(t4[:], nt[:], omz[:])
            nc.vector.tensor_add(hnew, t4[:], zh[:])

        # Raw pred layout: preds_raw[32j+b, t*64+c] = pred[b, t, 64j+c];
        # the host untangles (j,b) afterwards.  One 2D DMA per body keeps
        # the loop at a single HW-DGE queue (the back-edge drain and the
        # PE's LDWEIGHTS descriptor only support a few sync waits).
        with tc.For_i(0, steps, unroll,
                      hint_engines=(mybir.EngineType.PE,)) as iv:
            stage = sb.tile([128, unroll * PC], F32, tag="predstage")
            for s in range(unroll):
                step_body(iv + s, s, stage)
            nc.sync.dma_start(preds[:, bass.ds(iv * PC, unroll * PC)],
                              stage[:])


_CACHE = {}


def _get_nc(steps, unroll):
    key = (steps, unroll)
    if key in _CACHE:
        return _CACHE[key]
    nc = bacc.Bacc("TRN2", target_bir_lowering=False, debug=False,
                   enable_asserts=False, num_devices=NCORES)
    blob = nc.dram_tensor("blob", [128, BLOBW], FP16,
                          kind="ExternalInput").ap()
    idf = nc.dram_tensor("idf", [128, 128], F32, kind="ExternalInput").ap()
    preds = nc.dram_tensor("preds", [128, steps * PC], F32,
                           kind="ExternalOutput").ap()
    with tile.TileContext(nc) as tc:
        _emit(tc, nc, blob, idf, preds, steps, unroll)
    nc.compile()
    _CACHE[key] = nc
    return nc


def _pack(y0_batch, enc_w, enc_b, w_ih, w_hh, bias, bias_n, dec_w, dec_b):
    f = lambda x: np.ascontiguousarray(np.asarray(x, dtype=np.float32))
    y0_batch, enc_w, enc_b = f(y0_batch), f(enc_w), f(enc_b)
    w_ih, w_hh, bias, bias_n = f(w_ih), f(w_hh), f(bias), f(bias_n)
    dec_w, dec_b = f(dec_w), f(dec_b)

    W_r = w_ih[0:H] + w_hh[0:H]
    W_z = w_ih[H:2 * H] + w_hh[H:2 * H]
    W_ni = w_ih[2 * H:3 * H]
    W_nh = w_hh[2 * H:3 * H]

    wcols, bcols = [], []
    for j in range(NG):
        f0, f1 = j * FC, (j + 1) * FC
        p0, p1 = j * PC, (j + 1) * PC
        wcols += [W_r[f0:f1].T, W_z[f0:f1].T, W_nh[f0:f1].T,
                  dec_w[p0:p1].T, W_ni[f0:f1].T]
        bcols += [bias[f0:f1], bias[H + f0:H + f1], bias_n[f0:f1],
                  dec_b[p0:p1], bias[2 * H + f0:2 * H + f1]]

    base = np.zeros((128, BLOBW), np.float32)
    wbig = np.concatenate(wcols, axis=1)            # [1024, 4352]
    base[:, OFF_WS:OFF_ES] = (
        wbig.reshape(KT, 128, NG * GW).transpose(1, 0, 2).reshape(128, -1))
    ebig = np.concatenate(
        [enc_w[j * FC:(j + 1) * FC, :].T for j in range(NG)], axis=1)
    base[:, OFF_ES:OFF_Y0T] = (
        ebig.reshape(2, 128, H).transpose(1, 0, 2).reshape(128, -1))
    base[0, OFF_BR:OFF_EB] = np.concatenate(bcols)
    base[0, OFF_EB:OFF_ID] = np.concatenate(
        [enc_b[j * FC:(j + 1) * FC] for j in range(NG)])
    base[:, OFF_ID:OFF_ONES] = np.eye(128, dtype=np.float32)
    base[0, OFF_ONES:BLOBW] = 1.0

    idf = np.ascontiguousarray(np.eye(128, dtype=np.float32))
    in_maps = []
    for c in range(NCORES):
        bc = base.copy()
        y0t = y0_batch[c * BL:(c + 1) * BL].T       # [256, 32]
        bc[:, OFF_Y0T:OFF_BR] = (
            y0t.reshape(2, 128, BL).transpose(1, 0, 2).reshape(128, -1))
        in_maps.append(dict(blob=bc.astype(np.float16), idf=idf))
    return in_maps


def _run(inputs, steps=T, unroll=8, **run_kwargs):
    in_maps = _pack(
        inputs["y0_batch"], inputs["enc_w"], inputs["enc_b"], inputs["w_ih"],
        inputs["w_hh"], inputs["bias"], inputs["bias_n"], inputs["dec_w"],
        inputs["dec_b"])
    nc = _get_nc(steps, unroll)
    res = run_bass_kernel_spmd(nc, in_maps, core_ids=list(range(NCORES)),
                               **run_kwargs)
    # preds_raw[32j+b, t*64+c] -> [b, t, 64j+c]
    outs = []
    for r in res.results:
        raw = r["preds"].reshape(NG, BL, steps, PC)
        outs.append(np.ascontiguousarray(raw.transpose(1, 2, 0, 3))
                    .reshape(BL, steps, D))
    return np.concatenate(outs, axis=0), res


def kernel(ts=None, y0_batch=None, enc_w=None, enc_b=None, w_ih=None,
           w_hh=None, bias=None, bias_n=None, dec_w=None, dec_b=None):
    steps = int(np.asarray(ts).shape[0]) if ts is not None else T
    out, _ = _run(dict(y0_batch=y0_batch, enc_w=enc_w, enc_b=enc_b,
                       w_ih=w_ih, w_hh=w_hh, bias=bias, bias_n=bias_n,
                       dec_w=dec_w, dec_b=dec_b), steps=steps)
    return out
